# revision 31
# baseline (speedup 1.0000x reference)
"""Bass/Trainium2 kernel for fused bilinear attention + softmax.

reference computation:
    pa = a @ Wa + ba                      (B, La, D)
    pb = b @ Wb + bb                      (B, Lb, D)
    scores = einsum('bid,bjd->bij', pa * w, pb) + wbias
    out = softmax(scores.reshape(B, La*Lb)).reshape(B, La, Lb)

Device strategy (8 NeuronCores, data-parallel over batch, 8 batches/core):
    Weight-only host folding:  M = (Wa*w) @ Wb.T,  u = (Wa*w)@bb,  v = (Wb*w)@ba
      scores[b,i,j] = a_i M b_j^T + (a_i.u) + (v.b_j) + const
    const (+wbias) is dropped: softmax over the flattened grid is shift-invariant.
    The v.b_j term (bu) is rank-1 and input-cheap -> computed on host, applied
    on device as a per-partition ACT bias.

    Per pair of batches (rhs free dim 512), computed TRANSPOSED (S^T[j,i]):
      TT   = M @ bT + u        phase-1: hybrid fp8(e4m3, DoubleRow 2x) for the
                               first KF8 feature chunks + bf16 for the rest.
                               All operands pre-scaled by powers of 2 so fp8 and
                               bf16 partial products share one PSUM group:
                                 M*2^17 (fp8/bf16), b*2^5, a*2^5, u*2^22.
      S^T  = TT^T-chunks (stationary) x aT (moving), PSUM = 2^27 * S^T
      softmax: ACT exp with scale=2^-27 and bias=bu[j] (per-partition), fused
               rowsum (accum_out) -> GpSimd partition_all_reduce -> DVE
               reciprocal -> DVE scale -> contiguous DMA of S^T; host
               transposes back to S.
    PE warm-up matmuls run during the initial DMAs (HAM clock-gate release).
"""

import numpy as np
import ml_dtypes

import concourse.bass as bass
import concourse.bacc as bacc
import concourse.mybir as mybir
import concourse.tile as tile
from concourse.bass_utils import run_bass_kernel_spmd

BF16 = ml_dtypes.bfloat16
FP8 = ml_dtypes.float8_e4m3

N_CORES = 8
B, L, K = 64, 256, 1024          # batch, seq len (La=Lb), feature dim (IN_A=IN_B)
BPC = B // N_CORES               # batches per core
G = BPC // 2                     # batch-pair groups per core
KC = K // 128                    # feature chunks of 128
KF8 = 4                          # feature chunks computed in fp8 (of KC total)
KBF = KC - KF8                   # feature chunks computed in bf16
SC_M = float(2 ** 17)            # M scale
SC_AB = float(2 ** 5)            # a / b scale
SC_TT = float(2 ** 22)           # TT (and u) scale = SC_M * SC_AB
SC_S = float(2 ** 27)            # scores scale = SC_TT * SC_AB
F32 = mybir.dt.float32
DBF = mybir.dt.bfloat16
DF16 = mybir.dt.float16
DF8 = mybir.dt.float8e4
Act = mybir.ActivationFunctionType
DR = mybir.MatmulPerfMode.DoubleRow


def _build_program():
    # Bacc (not raw Bass): its compile() legalizes multi-wait instructions
    # (TRN2 allows at most one sync wait per instruction).
    nc = bacc.Bacc("TRN2", debug=False, target_bir_lowering=False)

    # Inputs keep l/m-blocked layouts so every per-l/per-m DMA call reads one
    # fully CONTIGUOUS DRAM region; calls round-robin over the 16 DMA engines
    # (~17 GB/s each), so per-block calls give both contiguity and parallelism.
    at = nc.dram_tensor("at", [G, K, 2 * L], DBF, kind="ExternalInput")
    bt8 = nc.dram_tensor("bt8", [G, KF8 * 128, 2 * L], DF8, kind="ExternalInput")
    btb = nc.dram_tensor("btb", [G, KBF * 128, 2 * L], DBF, kind="ExternalInput")
    # mt*[m, p, l, ki] = (M*2^17).T[l*128+p, m*128+ki]
    mt8 = nc.dram_tensor("mt8", [KC, 128, KF8, 128], DF8, kind="ExternalInput")
    mtb = nc.dram_tensor("mtb", [KC, 128, KBF, 128], DBF, kind="ExternalInput")
    us = nc.dram_tensor("us", [K], F32, kind="ExternalInput")     # u * 2^22
    buc = nc.dram_tensor("buc", [G * 4, 128], F32, kind="ExternalInput")
    # output as fp16 scaled by 2^10 (values ~1.5e-5 would be fp16-subnormal
    # unscaled); host divides by 1024 (exact). Halves output DMA bytes.
    probst = nc.dram_tensor("probst", [BPC, L, L], DF16, kind="ExternalOutput")

    with tile.TileContext(nc) as tc:
        with (
            tc.tile_pool(name="consts", bufs=1) as consts,
            tc.tile_pool(name="inp_at", bufs=3) as at_pool,
            tc.tile_pool(name="inp_bt8", bufs=3) as bt8_pool,
            tc.tile_pool(name="inp_btb", bufs=3) as btb_pool,
            tc.tile_pool(name="tt", bufs=10) as tt_pool,
            tc.tile_pool(name="sm", bufs=4) as sm_pool,
            tc.tile_pool(name="small", bufs=8) as small,
            tc.tile_pool(name="ps_tt", bufs=5, space="PSUM") as ps_tt,
            tc.tile_pool(name="ps_sc", bufs=2, space="PSUM") as ps_sc,
            tc.tile_pool(name="ps_w", bufs=1, space="PSUM") as ps_w,
        ):
            # ---- constants ----
            # u/bu are tiny and first needed at the first eviction/exp; issue
            # them on the ACT queue so the Sync queue starts on mt8/bt8.
            u_sb = consts.tile([128, KC], F32)              # u[c*128+p] at [p, c]
            nc.sync.dma_start(out=u_sb, in_=us[:].rearrange("(c p) -> p c", p=128))
            bu_sb = consts.tile([128, G * 4], F32)          # bu col per (g,q,jb)
            nc.sync.dma_start(out=bu_sb, in_=buc[:].rearrange("c p -> p c"))
            mt8_sb = consts.tile([128, KC, KF8, 128], DF8)  # [p, m, l, ki]
            mtb_sb = consts.tile([128, KC, KBF, 128], DBF)

            from concourse import library_config
            nc.gpsimd.load_library(library_config.attnmlp)

            # PE warm-up: dummy matmuls while the first DMAs land, so the HAM
            # clock gate is already released when real matmuls start.
            warm_sb = consts.tile([128, 2 * L], DBF)
            nc.vector.memset(warm_sb, 0.0)
            warm_ps = ps_w.tile([128, 2 * L], F32, tag="warm")
            for i in range(5):
                nc.tensor.matmul(
                    warm_ps, warm_sb[:, 0:128], warm_sb,
                    start=(i == 0), stop=(i == 4),
                )

            for g in range(G):
                bt8_sb = bt8_pool.tile([128, KF8, 2 * L], DF8, tag="bt8")
                btb_sb = btb_pool.tile([128, KBF, 2 * L], DBF, tag="btb")
                if g == 0:
                    nc.sync.dma_start(out=mt8_sb[:, 0], in_=mt8[0])
                for l in range(KF8):
                    nc.sync.dma_start(
                        out=bt8_sb[:, l], in_=bt8[g, l * 128 : (l + 1) * 128, :]
                    )
                if g == 0:
                    nc.sync.dma_start(out=mtb_sb[:, 0], in_=mtb[0])
                for l in range(KBF):
                    nc.sync.dma_start(
                        out=btb_sb[:, l], in_=btb[g, l * 128 : (l + 1) * 128, :]
                    )
                if g == 0:
                    for m in range(1, KC):
                        nc.sync.dma_start(out=mt8_sb[:, m], in_=mt8[m])
                        nc.sync.dma_start(out=mtb_sb[:, m], in_=mtb[m])
                at_sb = at_pool.tile([128, KC, 2 * L], DBF, tag="at")
                for l in range(KC):
                    nc.sync.dma_start(
                        out=at_sb[:, l], in_=at[g, l * 128 : (l + 1) * 128, :]
                    )

                # Phase 1: all 8 TT chunks (kept in SBUF; tt_pool holds them all)
                tt_chunks = []
                for m in range(KC):
                    tt_ps = ps_tt.tile([128, 2 * L], F32, tag="tt_ps")
                    for lp in range(KF8 // 2):
                        nc.tensor.matmul(
                            tt_ps,
                            mt8_sb[:, m, 2 * lp : 2 * lp + 2, :],
                            bt8_sb[:, 2 * lp : 2 * lp + 2, :],
                            start=(lp == 0), stop=False,
                            perf_mode=DR,
                        )
                    for lb in range(KBF):
                        nc.tensor.matmul(
                            tt_ps, mtb_sb[:, m, lb, :], btb_sb[:, lb, :],
                            start=False, stop=(lb == KBF - 1),
                        )
                    tt_sb = tt_pool.tile([128, 2 * L], DBF, tag="tt")
                    # TT' = TT + u[chunk m] (folds the a.u rank-1 term); DVE
                    # (not ACT) so the scalar engine never swaps LUT tables.
                    nc.vector.tensor_scalar_add(tt_sb, tt_ps, u_sb[:, m : m + 1])
                    tt_chunks.append(tt_sb)

                # Phase 2: S^T per batch in ONE psum bank (free = (jb, i)),
                # then fused exp+rowsum with bu as per-partition bias.
                for q in range(2):
                    sc_ps = ps_sc.tile([128, 2 * L], F32, tag="sc")
                    for jb in range(2):
                        for m in range(KC):
                            nc.tensor.matmul(
                                sc_ps[:, jb * L : (jb + 1) * L],
                                tt_chunks[m][
                                    :, q * L + jb * 128 : q * L + jb * 128 + 128
                                ],
                                at_sb[:, m, q * L : (q + 1) * L],
                                start=(m == 0), stop=(m == KC - 1),
                            )

                    # ---- softmax over the whole (256, 256) grid per batch ----
                    exp_sb = sm_pool.tile([128, 2 * L], F32, tag="exp")
                    cs = small.tile([128, 2], F32, tag="cs")
                    for jb in range(2):
                        idx = g * 4 + q * 2 + jb
                        nc.scalar.activation(
                            exp_sb[:, jb * L : (jb + 1) * L],
                            sc_ps[:, jb * L : (jb + 1) * L],
                            Act.Exp,
                            bias=bu_sb[:, idx : idx + 1],
                            scale=1.0 / SC_S,
                            accum_out=cs[:, jb : jb + 1],
                        )
                    colsum = small.tile([128, 1], F32, tag="colsum")
                    nc.vector.tensor_add(colsum, cs[:, 0:1], cs[:, 1:2])
                    # total over partitions, broadcast to all (GpSimd), recip
                    tot_col = small.tile([128, 1], F32, tag="totc")
                    nc.gpsimd.partition_all_reduce(
                        tot_col, colsum, channels=128,
                        reduce_op=bass.bass_isa.ReduceOp.add,
                    )
                    rcp_col = small.tile([128, 1], F32, tag="rcpc")
                    nc.vector.reciprocal(rcp_col, tot_col)
                    rcp_sc = small.tile([128, 1], F32, tag="rcpsc")
                    nc.vector.tensor_scalar_mul(rcp_sc, rcp_col, 1024.0)
                    probs_sb = sm_pool.tile([128, 2 * L], DF16, tag="probs")
                    for jb in range(2):
                        # split by half so the first DMA overlaps the second mul
                        nc.vector.tensor_scalar_mul(
                            probs_sb[:, jb * L : (jb + 1) * L],
                            exp_sb[:, jb * L : (jb + 1) * L],
                            rcp_sc,
                        )
                        nc.sync.dma_start(
                            out=probst[2 * g + q][jb * 128 : (jb + 1) * 128, :],
                            in_=probs_sb[:, jb * L : (jb + 1) * L],
                        )
    return nc


def _prep_host(a, b, Wa, ba, Wb, bb, w, wbias):
    """Weight folding (f64) + per-core feature-major scaled shards."""
    Wa64 = Wa.astype(np.float64)
    Wb64 = Wb.astype(np.float64)
    w64 = w.astype(np.float64)
    M = (Wa64 * w64[None, :]) @ Wb64.T                  # (K, K)
    u_np = (Wa64 * w64[None, :]) @ bb.astype(np.float64)
    v_np = (Wb64 * w64[None, :]) @ ba.astype(np.float64)
    # bu[b, j] = v . b_j  (rank-1 bias, input-cheap -> host)
    bu = np.einsum("k,bjk->bj", v_np, b.astype(np.float64))

    # m-major blocked (M*2^17)^T: y[m, p, l, ki] = Ms.T[l*128+p, m*128+ki]
    y = np.ascontiguousarray(
        (M * SC_M).T.reshape(KC, 128, KC, 128).transpose(2, 1, 0, 3)
    )
    mt8_np = np.clip(y[:, :, :KF8, :], -240, 240).astype(FP8)
    mtb_np = np.ascontiguousarray(y[:, :, KF8:, :]).astype(np.float32).astype(BF16)
    us_np = (u_np * SC_TT).astype(np.float32)

    def shard_f(x):
        # (BPC, L, K) -> (G, K, 2L) feature-major, batch pairs side by side
        xt = x.transpose(0, 2, 1)                        # (BPC, K, L)
        xt = xt.reshape(G, 2, K, L).transpose(0, 2, 1, 3).reshape(G, K, 2 * L)
        return np.ascontiguousarray(xt)

    in_maps = []
    for c in range(N_CORES):
        sl = slice(c * BPC, (c + 1) * BPC)
        zb = shard_f(b[sl].astype(np.float64) * SC_AB)
        za = shard_f(a[sl].astype(np.float64) * SC_AB)
        in_maps.append(
            {
                "at": za.astype(np.float32).astype(BF16),
                "bt8": np.clip(zb[:, : KF8 * 128, :], -240, 240).astype(FP8),
                "btb": np.ascontiguousarray(zb[:, KF8 * 128 :, :])
                .astype(np.float32)
                .astype(BF16),
                "mt8": mt8_np,
                "mtb": mtb_np,
                "us": us_np,
                "buc": np.ascontiguousarray(
                    bu[sl].reshape(G * 4, 128)
                ).astype(np.float32),
            }
        )
    return in_maps


def _postprocess(results):
    """Gather per-core transposed probs -> full (B, L, L) output."""
    return (
        np.concatenate(
            [results[c]["probst"].transpose(0, 2, 1) for c in range(N_CORES)],
            axis=0,
        ).astype(np.float32)
        * (1.0 / 1024.0)
    )


def _run(inputs, trace=False):
    nc = _build_program()
    nc.compile()
    in_maps = _prep_host(**inputs)
    res = run_bass_kernel_spmd(
        nc, in_maps, core_ids=list(range(N_CORES)), trace=trace
    )
    return _postprocess(res.results), res


def kernel(**inputs) -> np.ndarray:
    out, _ = _run(inputs, trace=False)
    return out
